# revision 1
# baseline (speedup 1.0000x reference)
"""Angular LSH bucketing kernel for 8 TRN2 NeuronCores.

Reference computation:
    scores  = mat @ proj_dir          # [b, h, n, 8]
    bits    = scores > 0
    bin_ids = sum(bits * 2^r)
    buckets = perm[bin_ids]           # perm is the Gray-code table

Sharding: data-parallel over batch*heads (64 -> 8 per core); proj/perm/enc
replicated (tiny).

Device strategy (per core, 65536 rows of 64 f32):
  - Host splits each f32 into hi+lo bf16 (a + b), viewed as [pairs, 128]
    bf16 (two consecutive 64-dim rows = one 128-deep column).
  - X-bar DMA-transpose loads [128, pairs] tiles at ~full HBM bandwidth.
  - 3 bf16 matmuls per 128-pair slice accumulate fp32 scores in PSUM
    (A*p_hi + A*p_lo + B*p_hi; B*p_lo term is below fp32 noise), using a
    block-diagonal projection so even/odd rows both get all 8 projections:
    out psum [128 pairs, (parity, proj)] -- rows on partitions.
  - Vector stage (wide [128, 512] ops): bits = scores > 0; for the Gray
    permutation perm[x] = x ^ (x >> 1), bucket bit r = bits_r XOR
    bits_{r+1} (bit 7 = bits_7), so bucket = sum_r (bits_r != bits_{r+1})
    * 2^r with the r=7 slot patched to bits_7 * 128. Grouped 8-wide
    reduce, cast to int32, DMA out.
  - If perm/enc_vec are not the Gray-code/power-of-two tables, falls back
    to computing bin_ids on device and applying perm on the host.
"""

import numpy as np
import ml_dtypes

from concourse import bass, mybir, tile
from concourse.bass_utils import run_bass_kernel_spmd

N_CORES = 8
B, H, N, D = 2, 32, 8192, 64
NPROJ = 8
ROWS_PER_CORE = (B * H // N_CORES) * N  # 65536
PAIRS = ROWS_PER_CORE // 2  # 32768
CHUNK_PAIRS = 4096
NCHUNK = PAIRS // CHUNK_PAIRS  # 8
U_PER_CHUNK = CHUNK_PAIRS // 128  # 32

F32 = mybir.dt.float32
BF16 = mybir.dt.bfloat16
I32 = mybir.dt.int32

_cache = {}



TAU = 0.06  # |score| threshold below which the host recomputes the row exactly


def _build_v3(gray: bool, pairs: int = PAIRS):
    """Natural full-bandwidth loads + PE transpose + ACT evacuation.

    Input is a single bf16 image of mat ("halves" pairing: column block j
    holds rows q + j*PAIRS), loaded contiguously at full HBM bandwidth.
    The X-bar transpose path is avoided entirely (measured ~115 GB/s packet
    ceiling); instead TensorE transposes [128,128] tiles through PSUM and
    ScalarE evacuates them. A per-row min|score| is emitted so the caller
    can recompute rows where bf16 rounding could flip a sign bit.
    """
    n_r = pairs // 128          # transpose tiles (r values)
    nchunk = n_r // 32          # DMA/psum chunks of 32 r each
    ngroups = n_r // 4          # transpose/evac groups of 4 r each
    fw = 512                    # psum free width per chunk (32 r * 16)
    nc = bass.Bass()
    a_d = nc.declare_dram_parameter("a", [pairs, 128], BF16, isOutput=False)
    pw_d = nc.declare_dram_parameter("pw", [128, 32], BF16, isOutput=False)
    w_d = nc.declare_dram_parameter("w", [128, fw], BF16, isOutput=False)
    id_d = nc.declare_dram_parameter("ident", [128, 128], BF16, isOutput=False)
    out_d = nc.declare_dram_parameter("out", [2 * pairs], I32, isOutput=True)
    ma_d = nc.declare_dram_parameter("ma", [2 * pairs], F32, isOutput=True)

    from contextlib import ExitStack

    with ExitStack() as ctx:
        ent = ctx.enter_context
        a_n = ent(nc.sbuf_tensor("a_n", [128, n_r, 128], BF16))
        pw_sb = ent(nc.sbuf_tensor("pw_sb", [128, 32], BF16))
        w_sb = ent(nc.sbuf_tensor("w_sb", [128, fw], BF16))
        id_sb = ent(nc.sbuf_tensor("id_sb", [128, 128], BF16))
        mt_sb = ent(nc.sbuf_tensor("mt_sb", [128, 4, 512], BF16))  # 4 group slots
        bt = ent(nc.sbuf_tensor("bt", [128, fw], F32))
        g = ent(nc.sbuf_tensor("g", [128, fw], F32))
        m = ent(nc.sbuf_tensor("m", [128, fw], F32))
        bf = ent(nc.sbuf_tensor("bf", [128, fw // 8], F32))
        fence_sb = ent(nc.sbuf_tensor("fence_sb", [128, 2], BF16))
        bi = ent(nc.sbuf_tensor("bi", [128, nchunk * fw // 8], I32))
        ma_sb = ent(nc.sbuf_tensor("ma_sb", [128, nchunk * fw // 8], F32))
        ps = ent(nc.psum_tensor("ps", [128, 3, fw], F32))       # 3 chunk slots
        scr = ent(nc.psum_tensor("scr", [128, 128], F32))       # fence scratch
        pst = ent(nc.psum_tensor("pst", [128, 4, 1024], BF16))  # 4 group slots, one bank each

        cs_sem = ent(nc.semaphore("cs_sem"))
        ch_sems = [ent(nc.semaphore(f"ch_sem{c}")) for c in range(nchunk)]
        pet_sem = ent(nc.semaphore("pet_sem"))    # transpose groups done (PE)
        act_sem = ent(nc.semaphore("act_sem"))    # evac groups done (ACT)
        pemm_sem = ent(nc.semaphore("pemm_sem"))  # MM groups done (PE)
        dve_sem = ent(nc.semaphore("dve_sem"))    # chunks done (DVE)
        dvrel_sem = ent(nc.semaphore("dvrel_sem"))  # chunk fence (PE dummy MM)
        out_sem = ent(nc.semaphore("out_sem"))

        def mm_group(tensor, gg):
            k = gg // 8
            if gg % 8 == 0 and k >= 3:
                tensor.wait_ge(dve_sem, k - 2)  # psum chunk slot reuse
            for i in range(4):
                r = 4 * gg + i
                ri = r % 32
                lhsT = mt_sb[:, gg % 4, 128 * i : 128 * (i + 1)]
                o = ps[:, k % 3, 16 * ri : 16 * (ri + 1)]
                tensor.matmul(o, lhsT, pw_sb[:, 0:16], start=True, stop=False)
                mm = tensor.matmul(
                    o, lhsT, pw_sb[:, 16:32], start=False, stop=True
                )
            mm.then_inc(pemm_sem, 1)
            if gg % 8 == 7:
                # fence: a dummy matmul whose 128-column fill outlasts the
                # prior matmul's PSUM drain; its inc releases the DVE read.
                tensor.matmul(
                    scr[0:16, :], pw_sb[:, 0:16], id_sb[:], start=True, stop=True
                ).then_inc(dvrel_sem, 1)

        with nc.Block() as block:

            @block.sync
            def _(sync):
                sync.dma_start(out=pw_sb[:], in_=pw_d[:]).then_inc(cs_sem, 16)
                sync.dma_start(out=w_sb[:], in_=w_d[:]).then_inc(cs_sem, 16)
                sync.dma_start(out=id_sb[:], in_=id_d[:]).then_inc(cs_sem, 16)
                a_view = a_d[:].rearrange("(P r) c -> P r c", P=128)
                for k in range(nchunk):
                    sync.dma_start(
                        out=a_n[:, 32 * k : 32 * (k + 1), :],
                        in_=a_view[:, 32 * k : 32 * (k + 1), :],
                    ).then_inc(ch_sems[k], 16)
                sync.wait_ge(dve_sem, nchunk)
                for k in range(nchunk):
                    csl = slice(k * fw // 8, (k + 1) * fw // 8)
                    dst = out_d[:].rearrange(
                        "(j P kk ri) -> P j kk ri", j=2, P=128, kk=nchunk
                    )[:, :, k, :]
                    sync.dma_start(
                        out=dst,
                        in_=bi[:, csl].rearrange("p (j ri) -> p j ri", j=2),
                    ).then_inc(out_sem, 16)
                    dst2 = ma_d[:].rearrange(
                        "(j P kk ri) -> P j kk ri", j=2, P=128, kk=nchunk
                    )[:, :, k, :]
                    sync.dma_start(
                        out=dst2,
                        in_=ma_sb[:, csl].rearrange("p (j ri) -> p j ri", j=2),
                    ).then_inc(out_sem, 16)
                sync.wait_ge(out_sem, 32 * nchunk)

            @block.tensor
            def _(tensor):
                tensor.wait_ge(cs_sem, 48)

                def t_group(gg):
                    k = gg // 8
                    if gg % 8 == 0:
                        tensor.wait_ge(ch_sems[k], 16)
                    for i in range(4):
                        r = 4 * gg + i
                        t = tensor.transpose(
                            pst[:, gg % 4, 128 * i : 128 * (i + 1)],
                            a_n[:, r, :],
                            id_sb[:],
                        )
                    t.then_inc(pet_sem, 4)

                # transposes run two groups ahead of the matmuls so the
                # scalar-engine evacuation pipelines instead of ping-ponging
                t_group(0)
                t_group(1)
                t_group(2)
                for gg in range(3, ngroups):
                    t_group(gg)
                    tensor.wait_ge(act_sem, gg - 2)
                    mm_group(tensor, gg - 3)
                for gg in range(ngroups - 3, ngroups):
                    tensor.wait_ge(act_sem, gg + 1)
                    mm_group(tensor, gg)

            @block.scalar
            def _(scalar):
                for gp in range(ngroups // 2):
                    g0 = 2 * gp  # evacuate groups g0, g0+1 in one pass
                    scalar.wait_ge(pet_sem, 4 * (g0 + 2))
                    if g0 >= 4:
                        scalar.wait_ge(pemm_sem, g0 - 2)
                    scalar.copy(
                        mt_sb[:, g0 % 4 : g0 % 4 + 2, :],
                        pst[:, g0 % 4 : g0 % 4 + 2, 0:512],
                    )
                    # fence: re-read the tail of the copy on the same engine
                    # so the inc cannot outrun the final SBUF writes
                    scalar.copy(
                        fence_sb[:], mt_sb[:, g0 % 4 + 1, 510:512]
                    ).then_inc(act_sem, 2)

            @block.vector
            def _(vector):
                vector.wait_ge(cs_sem, 48)
                for k in range(nchunk):
                    vector.wait_ge(dvrel_sem, k + 1)
                    psl = ps[:, k % 3, :]
                    vector.tensor_single_scalar(
                        bt[:], psl, 0.0, mybir.AluOpType.is_gt
                    )
                    if gray:
                        vector.tensor_tensor(
                            g[:, 0 : fw - 1],
                            bt[:, 0 : fw - 1],
                            bt[:, 1:fw],
                            op=mybir.AluOpType.not_equal,
                        )
                        vector.tensor_copy(g[:, 7:fw:8], bt[:, 7:fw:8])
                        msrc = g
                    else:
                        msrc = bt
                    vector.tensor_mul(m[:], msrc[:], w_sb[:])
                    csl = slice(k * fw // 8, (k + 1) * fw // 8)
                    vector.tensor_reduce(
                        bf[:].rearrange("p (j u) -> p u j", u=32),
                        m[:].rearrange("p (u r) -> p u r", r=8),
                        axis=mybir.AxisListType.X,
                        op=mybir.AluOpType.add,
                    )
                    vector.tensor_copy(bi[:, csl], bf[:])
                    vector.tensor_reduce(
                        ma_sb[:, csl].rearrange("p (j u) -> p u j", u=32),
                        psl.rearrange("p (u r) -> p u r", r=8),
                        axis=mybir.AxisListType.X,
                        op=mybir.AluOpType.min,
                        apply_absolute_value=True,
                    ).then_inc(dve_sem, 1)
    return nc


def _prep_v3(mat, proj_dir, enc_vec, gray):
    bf16 = ml_dtypes.bfloat16
    flat = np.ascontiguousarray(mat.reshape(B * H * N, D), dtype=np.float32)
    rows_per_core = ROWS_PER_CORE
    p = np.asarray(proj_dir, dtype=np.float32).reshape(D, NPROJ)
    pa = p.astype(bf16)
    pb = (p - pa.astype(np.float32)).astype(bf16)
    pw = np.zeros((128, 32), dtype=bf16)
    pw[0:64, 0:8] = pa
    pw[64:128, 8:16] = pa
    pw[0:64, 16:24] = pb
    pw[64:128, 24:32] = pb
    wrow = (
        np.array([1, 2, 4, 8, 16, 32, 64, 128], dtype=np.float32)
        if gray
        else np.asarray(enc_vec, dtype=np.float32).reshape(NPROJ)
    )
    w = np.tile(wrow, 64).reshape(1, 512).repeat(128, axis=0).astype(bf16)
    ident = np.eye(128, dtype=bf16)

    a16 = flat.astype(bf16)
    in_maps = []
    for i in range(N_CORES):
        sh = a16[i * rows_per_core : (i + 1) * rows_per_core]
        a = np.concatenate([sh[:PAIRS], sh[PAIRS:]], axis=1)  # [PAIRS, 128]
        in_maps.append(
            {
                "a": np.ascontiguousarray(a),
                "pw": pw,
                "w": np.ascontiguousarray(w),
                "ident": ident,
            }
        )
    return in_maps


def _build_raw(gray: bool, pairs: int = PAIRS, chunk_pairs: int = CHUNK_PAIRS):
    """Raw-bass (no Tile) build: explicit per-engine streams + semaphores.

    The nix-packaged walrus accepts at most ONE sync wait per instruction
    and has no legalizer pass, which rules out TileContext (its tail drain
    always carries one wait per active processor). Raw streams let every
    cross-engine edge be a standalone wait_ge.
    """
    nchunk = pairs // chunk_pairs
    u_per_chunk = chunk_pairs // 128
    fw = 16 * u_per_chunk  # psum/free width per chunk (512 for default)
    assert nchunk * fw <= 4096, "psum overflow"
    nc = bass.Bass()
    a_d = nc.declare_dram_parameter("a", [pairs, 128], BF16, isOutput=False)
    b_d = nc.declare_dram_parameter("b", [pairs, 128], BF16, isOutput=False)
    pw_d = nc.declare_dram_parameter("pw", [32, 128], BF16, isOutput=False)
    w_d = nc.declare_dram_parameter("w", [fw, 128], BF16, isOutput=False)
    out_d = nc.declare_dram_parameter(
        "out", [nchunk, 2 * chunk_pairs], I32, isOutput=True
    )

    from contextlib import ExitStack

    with ExitStack() as ctx:
        ent = ctx.enter_context
        a_sb = ent(nc.sbuf_tensor("a_sb", [128, pairs], BF16))
        b_sb = ent(nc.sbuf_tensor("b_sb", [128, pairs], BF16))
        pw_sb = ent(nc.sbuf_tensor("pw_sb", [128, 32], BF16))
        w_sb = ent(nc.sbuf_tensor("w_sb", [128, fw], BF16))
        bt = ent(nc.sbuf_tensor("bt", [128, fw], F32))
        g = ent(nc.sbuf_tensor("g", [128, fw], F32))
        m = ent(nc.sbuf_tensor("m", [128, fw], F32))
        bf = ent(nc.sbuf_tensor("bf", [128, fw // 8], F32))
        fence_sb = ent(nc.sbuf_tensor("fence_sb", [128, 2], BF16))
        bi = ent(nc.sbuf_tensor("bi", [128, nchunk * fw // 8], I32))
        ps = ent(nc.psum_tensor("ps", [128, nchunk * fw], F32))

        cs_sem = ent(nc.semaphore("cs_sem"))
        ch_sems = [ent(nc.semaphore(f"ch_sem{c}")) for c in range(nchunk)]
        mm_sem = ent(nc.semaphore("mm_sem"))
        dve_sem = ent(nc.semaphore("dve_sem"))
        out_sem = ent(nc.semaphore("out_sem"))

        with nc.Block() as block:

            @block.sync
            def _(sync):
                sync.dma_start(
                    out=pw_sb[:], in_=pw_d[:], transpose=True
                ).then_inc(cs_sem, 16)
                sync.dma_start(
                    out=w_sb[:], in_=w_d[:], transpose=True
                ).then_inc(cs_sem, 16)
                for c in range(nchunk):
                    sl = slice(c * chunk_pairs, (c + 1) * chunk_pairs)
                    sync.dma_start(
                        out=a_sb[:, sl], in_=a_d[sl, :], transpose=True
                    ).then_inc(ch_sems[c], 16)
                    sync.dma_start(
                        out=b_sb[:, sl], in_=b_d[sl, :], transpose=True
                    ).then_inc(ch_sems[c], 16)
                # outputs: only after ALL compute (also keeps the xbar
                # transpose mode quiesced before normal DMAs run)
                sync.wait_ge(dve_sem, nchunk)
                for c in range(nchunk):
                    sync.dma_start(
                        out=out_d[c].rearrange("(u p j) -> p u j", p=128, j=2),
                        in_=bi[:, c * fw // 8 : (c + 1) * fw // 8].rearrange(
                            "p (u j) -> p u j", j=2
                        ),
                    ).then_inc(out_sem, 16)
                sync.wait_ge(out_sem, 16 * nchunk)

            @block.tensor
            def _(tensor):
                tensor.wait_ge(cs_sem, 32)
                for c in range(nchunk):
                    tensor.wait_ge(ch_sems[c], 32)
                    for u in range(u_per_chunk):
                        col = c * chunk_pairs + u * 128
                        lhsA = a_sb[:, col : col + 128]
                        lhsB = b_sb[:, col : col + 128]
                        o = ps[:, c * fw + u * 16 : c * fw + (u + 1) * 16]
                        tensor.matmul(
                            o, lhsA, pw_sb[:, 0:16], start=True, stop=False
                        )
                        tensor.matmul(
                            o, lhsA, pw_sb[:, 16:32], start=False, stop=False
                        )
                        mm3 = tensor.matmul(
                            o, lhsB, pw_sb[:, 0:16], start=False, stop=True
                        )
                    mm3.then_inc(mm_sem, 1)

            @block.vector
            def _(vector):
                vector.wait_ge(cs_sem, 32)
                for c in range(nchunk):
                    vector.wait_ge(mm_sem, c + 1)
                    psl = ps[:, c * fw : (c + 1) * fw]
                    vector.tensor_single_scalar(
                        bt[:], psl, 0.0, mybir.AluOpType.is_gt
                    )
                    if gray:
                        vector.tensor_tensor(
                            g[:, 0 : fw - 1],
                            bt[:, 0 : fw - 1],
                            bt[:, 1:fw],
                            op=mybir.AluOpType.not_equal,
                        )
                        vector.tensor_copy(g[:, 7:fw:8], bt[:, 7:fw:8])
                        msrc = g
                    else:
                        msrc = bt
                    vector.tensor_mul(m[:], msrc[:], w_sb[:])
                    vector.tensor_reduce(
                        bf[:],
                        m[:].rearrange("p (k r) -> p k r", r=8),
                        axis=mybir.AxisListType.X,
                        op=mybir.AluOpType.add,
                    )
                    vector.tensor_copy(
                        bi[:, c * fw // 8 : (c + 1) * fw // 8], bf[:]
                    ).then_inc(dve_sem, 1)
    return nc


def _build(gray: bool, pairs: int = PAIRS, chunk_pairs: int = CHUNK_PAIRS):
    nchunk = pairs // chunk_pairs
    u_per_chunk = chunk_pairs // 128
    nc = bass.Bass()
    a_d = nc.declare_dram_parameter("a", [pairs, 128], BF16, isOutput=False)
    b_d = nc.declare_dram_parameter("b", [pairs, 128], BF16, isOutput=False)
    pw_d = nc.declare_dram_parameter("pw", [32, 128], BF16, isOutput=False)
    w_d = nc.declare_dram_parameter("w", [512, 128], BF16, isOutput=False)
    out_d = nc.declare_dram_parameter(
        "out", [nchunk, 2 * chunk_pairs], I32, isOutput=True
    )

    bis = []
    with tile.TileContext(nc) as tc:
        with (
            tc.tile_pool(name="const", bufs=1) as constp,
            tc.tile_pool(name="ab", bufs=8) as abp,
            tc.tile_pool(name="work", bufs=2) as workp,
            tc.tile_pool(name="outp", bufs=8) as outp,
            tc.tile_pool(name="psum", bufs=8, space=bass.MemorySpace.PSUM) as psump,
        ):
            pw_sb = constp.tile([128, 32], BF16)
            nc.sync.dma_start(out=pw_sb[:], in_=pw_d[:], transpose=True)
            w_sb = constp.tile([128, 512], BF16)
            nc.sync.dma_start(out=w_sb[:], in_=w_d[:], transpose=True)
            # Touch w_sb on DVE once so later tensor_tensor ops don't each
            # need a DMA-queue wait (the DVE TT ISA slot allows only one
            # sync wait; walrus rejects two).
            w_touch = constp.tile([128, 1], F32)  # dtype-cast copy is fine
            nc.vector.tensor_copy(w_touch[:], w_sb[:, 0:1])

            for c in range(nchunk):
                sl = slice(c * chunk_pairs, (c + 1) * chunk_pairs)
                a_sb = abp.tile([128, chunk_pairs], BF16, tag="a")
                nc.sync.dma_start(out=a_sb[:], in_=a_d[sl, :], transpose=True)
                b_sb = abp.tile([128, chunk_pairs], BF16, tag="b")
                nc.sync.dma_start(out=b_sb[:], in_=b_d[sl, :], transpose=True)

                fw = 16 * u_per_chunk
                ps = psump.tile([128, fw], F32)
                for u in range(u_per_chunk):
                    lhsA = a_sb[:, u * 128 : (u + 1) * 128]
                    lhsB = b_sb[:, u * 128 : (u + 1) * 128]
                    o = ps[:, u * 16 : (u + 1) * 16]
                    nc.tensor.matmul(o, lhsA, pw_sb[:, 0:16], start=True, stop=False)
                    nc.tensor.matmul(o, lhsA, pw_sb[:, 16:32], start=False, stop=False)
                    nc.tensor.matmul(o, lhsB, pw_sb[:, 0:16], start=False, stop=True)

                bt = workp.tile([128, fw], F32, tag="bt")
                nc.vector.tensor_single_scalar(
                    bt[:], ps[:], 0.0, mybir.AluOpType.is_gt
                )
                if gray:
                    g = workp.tile([128, fw], F32, tag="g")
                    nc.vector.tensor_tensor(
                        g[:, 0 : fw - 1],
                        bt[:, 0 : fw - 1],
                        bt[:, 1:fw],
                        op=mybir.AluOpType.not_equal,
                    )
                    nc.vector.tensor_copy(g[:, 7:fw:8], bt[:, 7:fw:8])
                    msrc = g
                else:
                    msrc = bt
                m = workp.tile([128, fw], F32, tag="m")
                nc.vector.tensor_mul(m[:], msrc[:], w_sb[:, 0:fw])
                bf = workp.tile([128, fw // 8], F32, tag="bf")
                nc.vector.tensor_reduce(
                    bf[:],
                    m[:].rearrange("p (k r) -> p k r", r=8),
                    axis=mybir.AxisListType.X,
                    op=mybir.AluOpType.add,
                )
                bi = outp.tile([128, fw // 8], I32, tag=f"bi{c}")
                nc.vector.tensor_copy(bi[:], bf[:])
                bis.append(bi)
    # Past the TileContext exit: Tile has drained every engine and run an
    # all-engine barrier, so raw output DMAs here need no sync waits at
    # all (the DMA ISA slot only has one wait; inside the context the
    # xbar-transpose serialization would force 2+). Fence completion with
    # one semaphore.
    post = []
    with nc.semaphore("out_sem") as out_sem:
        for c, bi in enumerate(bis):
            h = nc.sync.dma_start(
                out=out_d[c].rearrange("(u p j) -> p u j", p=128, j=2),
                in_=bi[:].rearrange("p (u j) -> p u j", j=2),
            )
            h.then_inc(out_sem, 16)
            post.append(h)
        nc.sync.wait_ge(out_sem, 16 * len(bis))

    # Tile lowered only the instructions it traced; resolve the symbolic
    # tile APs on the raw post-context DMAs the same way tile.py does.
    def _concrete(arg):
        t = arg.bass_ap.tensor
        if hasattr(t, "concrete_tensor"):
            arg.bass_ap.tensor = t.concrete_tensor()
        return arg.bass_ap

    eng = nc.sync
    for h in post:
        inst = h.ins
        inst.ins, inst.outs = eng.lower_symbolic_args(
            inst.ins, inst.outs, _concrete, inst.debug
        )
    return nc


def _prep(mat, proj_dir, perm, enc_vec, gray):
    bf16 = ml_dtypes.bfloat16
    flat = np.ascontiguousarray(mat.reshape(B * H, N, D), dtype=np.float32)
    a_full = flat.astype(bf16)
    b_full = (flat - a_full.astype(np.float32)).astype(bf16)

    p = np.asarray(proj_dir, dtype=np.float32).reshape(D, NPROJ)
    pa = p.astype(bf16)
    pb = (p - pa.astype(np.float32)).astype(bf16)
    pw = np.zeros((128, 32), dtype=bf16)
    pw[0:64, 0:8] = pa
    pw[64:128, 8:16] = pa
    pw[0:64, 16:24] = pb
    pw[64:128, 24:32] = pb
    pw = np.ascontiguousarray(pw.T)  # shipped transposed; X-bar restores

    enc = np.asarray(enc_vec, dtype=np.float32).reshape(NPROJ)
    w = np.tile(enc, 64).reshape(1, 512).repeat(128, axis=0)
    if gray:
        # weights for the XOR-of-adjacent-bits formulation: 2^r for r<7,
        # 128 on the patched r=7 slot -- identical to enc for enc=2^r.
        w = np.tile(
            np.array([1, 2, 4, 8, 16, 32, 64, 128], dtype=np.float32), 64
        ).reshape(1, 512).repeat(128, axis=0)
    w = np.ascontiguousarray(w.T, dtype=np.float32).astype(bf16)

    bh_per_core = B * H // N_CORES
    in_maps = []
    for i in range(N_CORES):
        sh = a_full[i * bh_per_core : (i + 1) * bh_per_core]
        shb = b_full[i * bh_per_core : (i + 1) * bh_per_core]
        in_maps.append(
            {
                "a": np.ascontiguousarray(sh).reshape(PAIRS, 128),
                "b": np.ascontiguousarray(shb).reshape(PAIRS, 128),
                "pw": pw,
                "w": w,
            }
        )
    return in_maps



def _build_v4(gray: bool, pairs: int = PAIRS, chunk_pairs: int = CHUNK_PAIRS):
    """a-only variant of the xbar kernel: halves input DMA (the packet-rate
    bottleneck). Emits per-row min|score| so the host exactly recomputes
    rows inside the bf16 rounding envelope."""
    nchunk = pairs // chunk_pairs
    u_per_chunk = chunk_pairs // 128
    fw = 16 * u_per_chunk
    assert nchunk * fw <= 4096, "psum overflow"
    nc = bass.Bass()
    a_d = nc.declare_dram_parameter("a", [pairs, 128], BF16, isOutput=False)
    pw_d = nc.declare_dram_parameter("pw", [32, 128], BF16, isOutput=False)
    w_d = nc.declare_dram_parameter("w", [fw, 128], BF16, isOutput=False)
    out_d = nc.declare_dram_parameter(
        "out", [nchunk, 2 * chunk_pairs], I32, isOutput=True
    )

    from contextlib import ExitStack

    with ExitStack() as ctx:
        ent = ctx.enter_context
        a_sb = ent(nc.sbuf_tensor("a_sb", [128, pairs], BF16))
        pw_sb = ent(nc.sbuf_tensor("pw_sb", [128, 32], BF16))
        w_sb = ent(nc.sbuf_tensor("w_sb", [128, fw], BF16))
        bt = ent(nc.sbuf_tensor("bt", [128, fw], F32))
        g = ent(nc.sbuf_tensor("g", [128, fw], F32))
        m = ent(nc.sbuf_tensor("m", [128, fw], F32))
        bf = ent(nc.sbuf_tensor("bf", [128, fw // 8], F32))
        flg = ent(nc.sbuf_tensor("flg", [128, fw // 8], F32))
        bi = ent(nc.sbuf_tensor("bi", [128, nchunk * fw // 8], I32))
        ma_sb = ent(nc.sbuf_tensor("ma_sb", [128, nchunk * fw // 8], F32))
        ps = ent(nc.psum_tensor("ps", [128, nchunk * fw], F32))

        cs_sem = ent(nc.semaphore("cs_sem"))
        ch_sems = [ent(nc.semaphore(f"ch_sem{c}")) for c in range(nchunk)]
        mm_sem = ent(nc.semaphore("mm_sem"))
        dve_sem = ent(nc.semaphore("dve_sem"))
        out_sem = ent(nc.semaphore("out_sem"))

        with nc.Block() as block:

            @block.sync
            def _(sync):
                sync.dma_start(
                    out=pw_sb[:], in_=pw_d[:], transpose=True
                ).then_inc(cs_sem, 16)
                sync.dma_start(
                    out=w_sb[:], in_=w_d[:], transpose=True
                ).then_inc(cs_sem, 16)
                for c in range(nchunk):
                    sl = slice(c * chunk_pairs, (c + 1) * chunk_pairs)
                    sync.dma_start(
                        out=a_sb[:, sl], in_=a_d[sl, :], transpose=True
                    ).then_inc(ch_sems[c], 16)
                sync.wait_ge(dve_sem, nchunk)
                for c in range(nchunk):
                    csl = slice(c * fw // 8, (c + 1) * fw // 8)
                    sync.dma_start(
                        out=out_d[c].rearrange("(u p j) -> p u j", p=128, j=2),
                        in_=bi[:, csl].rearrange("p (u j) -> p u j", j=2),
                    ).then_inc(out_sem, 16)
                sync.wait_ge(out_sem, 16 * nchunk)

            @block.tensor
            def _(tensor):
                tensor.wait_ge(cs_sem, 32)
                for c in range(nchunk):
                    tensor.wait_ge(ch_sems[c], 16)
                    for u in range(u_per_chunk):
                        col = c * chunk_pairs + u * 128
                        lhsA = a_sb[:, col : col + 128]
                        o = ps[:, c * fw + u * 16 : c * fw + (u + 1) * 16]
                        tensor.matmul(
                            o, lhsA, pw_sb[:, 0:16], start=True, stop=False
                        )
                        mm = tensor.matmul(
                            o, lhsA, pw_sb[:, 16:32], start=False, stop=True
                        )
                    mm.then_inc(mm_sem, 1)

            @block.vector
            def _(vector):
                vector.wait_ge(cs_sem, 32)
                for c in range(nchunk):
                    vector.wait_ge(mm_sem, c + 1)
                    psl = ps[:, c * fw : (c + 1) * fw]
                    csl = slice(c * fw // 8, (c + 1) * fw // 8)
                    vector.tensor_single_scalar(
                        bt[:], psl, 0.0, mybir.AluOpType.is_gt
                    )
                    if gray:
                        vector.tensor_tensor(
                            g[:, 0 : fw - 1],
                            bt[:, 0 : fw - 1],
                            bt[:, 1:fw],
                            op=mybir.AluOpType.not_equal,
                        )
                        vector.tensor_copy(g[:, 7:fw:8], bt[:, 7:fw:8])
                        msrc = g
                    else:
                        msrc = bt
                    vector.tensor_mul(m[:], msrc[:], w_sb[:])
                    vector.tensor_reduce(
                        bf[:],
                        m[:].rearrange("p (k r) -> p k r", r=8),
                        axis=mybir.AxisListType.X,
                        op=mybir.AluOpType.add,
                    )
                    vector.tensor_reduce(
                        ma_sb[:, csl],
                        psl.rearrange("p (k r) -> p k r", r=8),
                        axis=mybir.AxisListType.X,
                        op=mybir.AluOpType.min,
                        apply_absolute_value=True,
                    )
                    # fuse the "needs host recompute" flag into bit 8 of the
                    # bucket word: out = bucket + 256*(min|score| < TAU)
                    vector.tensor_scalar(
                        flg[:], ma_sb[:, csl], TAU, 256.0,
                        mybir.AluOpType.is_lt, mybir.AluOpType.mult,
                    )
                    vector.tensor_add(bf[:], bf[:], flg[:])
                    vector.tensor_copy(bi[:, csl], bf[:]).then_inc(dve_sem, 1)
    return nc


def _prep_v4(mat, proj_dir, enc_vec, gray):
    bf16 = ml_dtypes.bfloat16
    flat = np.ascontiguousarray(mat.reshape(B * H, N, D), dtype=np.float32)
    a_full = flat.astype(bf16)

    p = np.asarray(proj_dir, dtype=np.float32).reshape(D, NPROJ)
    pa = p.astype(bf16)
    pb = (p - pa.astype(np.float32)).astype(bf16)
    pw = np.zeros((128, 32), dtype=bf16)
    pw[0:64, 0:8] = pa
    pw[64:128, 8:16] = pa
    pw[0:64, 16:24] = pb
    pw[64:128, 24:32] = pb
    pw = np.ascontiguousarray(pw.T)

    wrow = (
        np.array([1, 2, 4, 8, 16, 32, 64, 128], dtype=np.float32)
        if gray
        else np.asarray(enc_vec, dtype=np.float32).reshape(NPROJ)
    )
    w = np.tile(wrow, 64).reshape(1, 512).repeat(128, axis=0)
    w = np.ascontiguousarray(w.T).astype(bf16)

    bh_per_core = B * H // N_CORES
    in_maps = []
    for i in range(N_CORES):
        sh = a_full[i * bh_per_core : (i + 1) * bh_per_core]
        in_maps.append(
            {
                "a": np.ascontiguousarray(sh).reshape(PAIRS, 128),
                "pw": pw,
                "w": w,
            }
        )
    return in_maps


def _is_gray_setup(perm, enc_vec):
    perm = np.asarray(perm).reshape(-1)
    enc = np.asarray(enc_vec).reshape(-1)
    if perm.shape[0] != 256 or enc.shape[0] != NPROJ:
        return False
    idx = np.arange(256, dtype=np.int64)
    return bool(
        np.array_equal(perm, idx ^ (idx >> 1)) and np.array_equal(enc, 2 ** np.arange(8))
    )


def kernel(mat, proj_dir, perm, enc_vec, _trace=False, _tmpdir=None):
    gray = _is_gray_setup(perm, enc_vec)
    key = ("v4", gray)
    if key not in _cache:
        _cache[key] = _build_v4(gray)
    nc = _cache[key]

    in_maps = _prep_v4(mat, proj_dir, enc_vec, gray)
    res = run_bass_kernel_spmd(
        nc, in_maps, core_ids=list(range(N_CORES)), trace=_trace, tmpdir=_tmpdir
    )
    word = np.concatenate(
        [np.asarray(r["out"]).reshape(ROWS_PER_CORE) for r in res.results]
    ).astype(np.int64)
    buckets = word & 255          # bucket (or raw bin id if not gray)
    flagged = word >= 256         # device min|score| < TAU

    # Host fix-up: rows whose smallest |bf16 score| is inside the rounding
    # envelope get recomputed exactly.
    flat = np.ascontiguousarray(mat.reshape(B * H * N, D), dtype=np.float32)
    p = np.asarray(proj_dir, dtype=np.float32).reshape(D, NPROJ)
    enc = np.asarray(enc_vec).reshape(NPROJ).astype(np.int64)
    perm_arr = np.asarray(perm).reshape(-1).astype(np.int64)
    if not gray:
        buckets = perm_arr[buckets]  # device emitted raw bin ids
    idx = np.nonzero(flagged)[0]
    if idx.size:
        sc = flat[idx] @ p
        bits = (sc > 0).astype(np.int64)
        bins = (bits * enc).sum(-1)
        buckets[idx] = (bins ^ (bins >> 1)) if gray else perm_arr[bins]
    out = buckets.reshape(B, H, N).astype(np.int32)
    if _trace:
        return out, res
    return out



# revision 19
# speedup vs baseline: 2.8279x; 2.8279x over previous
"""Angular LSH bucketing kernel for 8 TRN2 NeuronCores.

Reference computation:
    scores  = mat @ proj_dir          # [b, h, n, 8]
    bits    = scores > 0
    bin_ids = sum(bits * 2^r)
    buckets = perm[bin_ids]           # perm is the Gray-code table

Sharding: data-parallel over batch*heads (64 -> 8 per core); proj/perm/enc
replicated (tiny).

Device strategy (per core, 65536 rows of 64 f32):
  - Host splits each f32 into hi+lo bf16 (a + b), viewed as [pairs, 128]
    bf16 (two consecutive 64-dim rows = one 128-deep column).
  - X-bar DMA-transpose loads [128, pairs] tiles at ~full HBM bandwidth.
  - 3 bf16 matmuls per 128-pair slice accumulate fp32 scores in PSUM
    (A*p_hi + A*p_lo + B*p_hi; B*p_lo term is below fp32 noise), using a
    block-diagonal projection so even/odd rows both get all 8 projections:
    out psum [128 pairs, (parity, proj)] -- rows on partitions.
  - Vector stage (wide [128, 512] ops): bits = scores > 0; for the Gray
    permutation perm[x] = x ^ (x >> 1), bucket bit r = bits_r XOR
    bits_{r+1} (bit 7 = bits_7), so bucket = sum_r (bits_r != bits_{r+1})
    * 2^r with the r=7 slot patched to bits_7 * 128. Grouped 8-wide
    reduce, cast to int32, DMA out.
  - If perm/enc_vec are not the Gray-code/power-of-two tables, falls back
    to computing bin_ids on device and applying perm on the host.
"""

import numpy as np
import ml_dtypes

from concourse import bass, mybir, tile
from concourse.bass_utils import run_bass_kernel_spmd

N_CORES = 8
B, H, N, D = 2, 32, 8192, 64
NPROJ = 8
ROWS_PER_CORE = (B * H // N_CORES) * N  # 65536
PAIRS = ROWS_PER_CORE // 2  # 32768
CHUNK_PAIRS = 4096
NCHUNK = PAIRS // CHUNK_PAIRS  # 8
U_PER_CHUNK = CHUNK_PAIRS // 128  # 32

F32 = mybir.dt.float32
BF16 = mybir.dt.bfloat16
I32 = mybir.dt.int32

_cache = {}



TAU = 0.06  # |score| threshold below which the host recomputes the row exactly


def _build_v3(gray: bool, pairs: int = PAIRS):
    """Natural full-bandwidth loads + PE transpose + ACT evacuation.

    Input is a single bf16 image of mat ("halves" pairing: column block j
    holds rows q + j*PAIRS), loaded contiguously at full HBM bandwidth.
    The X-bar transpose path is avoided entirely (measured ~115 GB/s packet
    ceiling); instead TensorE transposes [128,128] tiles through PSUM and
    ScalarE evacuates them. A per-row min|score| is emitted so the caller
    can recompute rows where bf16 rounding could flip a sign bit.
    """
    n_r = pairs // 128          # transpose tiles (r values)
    nchunk = n_r // 32          # DMA/psum chunks of 32 r each
    ngroups = n_r // 4          # transpose/evac groups of 4 r each
    fw = 512                    # psum free width per chunk (32 r * 16)
    nc = bass.Bass()
    a_d = nc.declare_dram_parameter("a", [pairs, 128], BF16, isOutput=False)
    pw_d = nc.declare_dram_parameter("pw", [128, 32], BF16, isOutput=False)
    w_d = nc.declare_dram_parameter("w", [128, fw], BF16, isOutput=False)
    id_d = nc.declare_dram_parameter("ident", [128, 128], BF16, isOutput=False)
    out_d = nc.declare_dram_parameter("out", [2 * pairs], I32, isOutput=True)
    ma_d = nc.declare_dram_parameter("ma", [2 * pairs], F32, isOutput=True)

    from contextlib import ExitStack

    with ExitStack() as ctx:
        ent = ctx.enter_context
        a_n = ent(nc.sbuf_tensor("a_n", [128, n_r, 128], BF16))
        pw_sb = ent(nc.sbuf_tensor("pw_sb", [128, 32], BF16))
        w_sb = ent(nc.sbuf_tensor("w_sb", [128, fw], BF16))
        id_sb = ent(nc.sbuf_tensor("id_sb", [128, 128], BF16))
        mt_sb = ent(nc.sbuf_tensor("mt_sb", [128, 4, 512], BF16))  # 4 group slots
        bt = ent(nc.sbuf_tensor("bt", [128, fw], F32))
        g = ent(nc.sbuf_tensor("g", [128, fw], F32))
        m = ent(nc.sbuf_tensor("m", [128, fw], F32))
        bf = ent(nc.sbuf_tensor("bf", [128, fw // 8], F32))
        fence_sb = ent(nc.sbuf_tensor("fence_sb", [128, 2], BF16))
        bi = ent(nc.sbuf_tensor("bi", [128, nchunk * fw // 8], I32))
        ma_sb = ent(nc.sbuf_tensor("ma_sb", [128, nchunk * fw // 8], F32))
        ps = ent(nc.psum_tensor("ps", [128, 3, fw], F32))       # 3 chunk slots
        scr = ent(nc.psum_tensor("scr", [128, 128], F32))       # fence scratch
        pst = ent(nc.psum_tensor("pst", [128, 4, 1024], BF16))  # 4 group slots, one bank each

        cs_sem = ent(nc.semaphore("cs_sem"))
        ch_sems = [ent(nc.semaphore(f"ch_sem{c}")) for c in range(nchunk)]
        pet_sem = ent(nc.semaphore("pet_sem"))    # transpose groups done (PE)
        act_sem = ent(nc.semaphore("act_sem"))    # evac groups done (ACT)
        pemm_sem = ent(nc.semaphore("pemm_sem"))  # MM groups done (PE)
        dve_sem = ent(nc.semaphore("dve_sem"))    # chunks done (DVE)
        dvrel_sem = ent(nc.semaphore("dvrel_sem"))  # chunk fence (PE dummy MM)
        out_sem = ent(nc.semaphore("out_sem"))

        def mm_group(tensor, gg):
            k = gg // 8
            if gg % 8 == 0 and k >= 3:
                tensor.wait_ge(dve_sem, k - 2)  # psum chunk slot reuse
            for i in range(4):
                r = 4 * gg + i
                ri = r % 32
                lhsT = mt_sb[:, gg % 4, 128 * i : 128 * (i + 1)]
                o = ps[:, k % 3, 16 * ri : 16 * (ri + 1)]
                tensor.matmul(o, lhsT, pw_sb[:, 0:16], start=True, stop=False)
                mm = tensor.matmul(
                    o, lhsT, pw_sb[:, 16:32], start=False, stop=True
                )
            mm.then_inc(pemm_sem, 1)
            if gg % 8 == 7:
                # fence: a dummy matmul whose 128-column fill outlasts the
                # prior matmul's PSUM drain; its inc releases the DVE read.
                tensor.matmul(
                    scr[0:16, :], pw_sb[:, 0:16], id_sb[:], start=True, stop=True
                ).then_inc(dvrel_sem, 1)

        with nc.Block() as block:

            @block.sync
            def _(sync):
                sync.dma_start(out=pw_sb[:], in_=pw_d[:]).then_inc(cs_sem, 16)
                sync.dma_start(out=w_sb[:], in_=w_d[:]).then_inc(cs_sem, 16)
                sync.dma_start(out=id_sb[:], in_=id_d[:]).then_inc(cs_sem, 16)
                a_view = a_d[:].rearrange("(P r) c -> P r c", P=128)
                for k in range(nchunk):
                    sync.dma_start(
                        out=a_n[:, 32 * k : 32 * (k + 1), :],
                        in_=a_view[:, 32 * k : 32 * (k + 1), :],
                    ).then_inc(ch_sems[k], 16)
                sync.wait_ge(dve_sem, nchunk)
                for k in range(nchunk):
                    csl = slice(k * fw // 8, (k + 1) * fw // 8)
                    dst = out_d[:].rearrange(
                        "(j P kk ri) -> P j kk ri", j=2, P=128, kk=nchunk
                    )[:, :, k, :]
                    sync.dma_start(
                        out=dst,
                        in_=bi[:, csl].rearrange("p (j ri) -> p j ri", j=2),
                    ).then_inc(out_sem, 16)
                    dst2 = ma_d[:].rearrange(
                        "(j P kk ri) -> P j kk ri", j=2, P=128, kk=nchunk
                    )[:, :, k, :]
                    sync.dma_start(
                        out=dst2,
                        in_=ma_sb[:, csl].rearrange("p (j ri) -> p j ri", j=2),
                    ).then_inc(out_sem, 16)
                sync.wait_ge(out_sem, 32 * nchunk)

            @block.tensor
            def _(tensor):
                tensor.wait_ge(cs_sem, 48)

                def t_group(gg):
                    k = gg // 8
                    if gg % 8 == 0:
                        tensor.wait_ge(ch_sems[k], 16)
                    for i in range(4):
                        r = 4 * gg + i
                        t = tensor.transpose(
                            pst[:, gg % 4, 128 * i : 128 * (i + 1)],
                            a_n[:, r, :],
                            id_sb[:],
                        )
                    t.then_inc(pet_sem, 4)

                # transposes run two groups ahead of the matmuls so the
                # scalar-engine evacuation pipelines instead of ping-ponging
                t_group(0)
                t_group(1)
                t_group(2)
                for gg in range(3, ngroups):
                    t_group(gg)
                    tensor.wait_ge(act_sem, gg - 2)
                    mm_group(tensor, gg - 3)
                for gg in range(ngroups - 3, ngroups):
                    tensor.wait_ge(act_sem, gg + 1)
                    mm_group(tensor, gg)

            @block.scalar
            def _(scalar):
                for gp in range(ngroups // 2):
                    g0 = 2 * gp  # evacuate groups g0, g0+1 in one pass
                    scalar.wait_ge(pet_sem, 4 * (g0 + 2))
                    if g0 >= 4:
                        scalar.wait_ge(pemm_sem, g0 - 2)
                    scalar.copy(
                        mt_sb[:, g0 % 4 : g0 % 4 + 2, :],
                        pst[:, g0 % 4 : g0 % 4 + 2, 0:512],
                    )
                    # fence: re-read the tail of the copy on the same engine
                    # so the inc cannot outrun the final SBUF writes
                    scalar.copy(
                        fence_sb[:], mt_sb[:, g0 % 4 + 1, 510:512]
                    ).then_inc(act_sem, 2)

            @block.vector
            def _(vector):
                vector.wait_ge(cs_sem, 48)
                for k in range(nchunk):
                    vector.wait_ge(dvrel_sem, k + 1)
                    psl = ps[:, k % 3, :]
                    vector.tensor_single_scalar(
                        bt[:], psl, 0.0, mybir.AluOpType.is_gt
                    )
                    if gray:
                        vector.tensor_tensor(
                            g[:, 0 : fw - 1],
                            bt[:, 0 : fw - 1],
                            bt[:, 1:fw],
                            op=mybir.AluOpType.not_equal,
                        )
                        vector.tensor_copy(g[:, 7:fw:8], bt[:, 7:fw:8])
                        msrc = g
                    else:
                        msrc = bt
                    vector.tensor_mul(m[:], msrc[:], w_sb[:])
                    csl = slice(k * fw // 8, (k + 1) * fw // 8)
                    vector.tensor_reduce(
                        bf[:].rearrange("p (j u) -> p u j", u=32),
                        m[:].rearrange("p (u r) -> p u r", r=8),
                        axis=mybir.AxisListType.X,
                        op=mybir.AluOpType.add,
                    )
                    vector.tensor_copy(bi[:, csl], bf[:])
                    vector.tensor_reduce(
                        ma_sb[:, csl].rearrange("p (j u) -> p u j", u=32),
                        psl.rearrange("p (u r) -> p u r", r=8),
                        axis=mybir.AxisListType.X,
                        op=mybir.AluOpType.min,
                        apply_absolute_value=True,
                    ).then_inc(dve_sem, 1)
    return nc


def _prep_v3(mat, proj_dir, enc_vec, gray):
    bf16 = ml_dtypes.bfloat16
    flat = np.ascontiguousarray(mat.reshape(B * H * N, D), dtype=np.float32)
    rows_per_core = ROWS_PER_CORE
    p = np.asarray(proj_dir, dtype=np.float32).reshape(D, NPROJ)
    pa = p.astype(bf16)
    pb = (p - pa.astype(np.float32)).astype(bf16)
    pw = np.zeros((128, 32), dtype=bf16)
    pw[0:64, 0:8] = pa
    pw[64:128, 8:16] = pa
    pw[0:64, 16:24] = pb
    pw[64:128, 24:32] = pb
    wrow = (
        np.array([1, 2, 4, 8, 16, 32, 64, 128], dtype=np.float32)
        if gray
        else np.asarray(enc_vec, dtype=np.float32).reshape(NPROJ)
    )
    w = np.tile(wrow, 64).reshape(1, 512).repeat(128, axis=0).astype(bf16)
    ident = np.eye(128, dtype=bf16)

    a16 = flat.astype(bf16)
    in_maps = []
    for i in range(N_CORES):
        sh = a16[i * rows_per_core : (i + 1) * rows_per_core]
        a = np.concatenate([sh[:PAIRS], sh[PAIRS:]], axis=1)  # [PAIRS, 128]
        in_maps.append(
            {
                "a": np.ascontiguousarray(a),
                "pw": pw,
                "w": np.ascontiguousarray(w),
                "ident": ident,
            }
        )
    return in_maps


def _build_raw(gray: bool, pairs: int = PAIRS, chunk_pairs: int = CHUNK_PAIRS):
    """Raw-bass (no Tile) build: explicit per-engine streams + semaphores.

    The nix-packaged walrus accepts at most ONE sync wait per instruction
    and has no legalizer pass, which rules out TileContext (its tail drain
    always carries one wait per active processor). Raw streams let every
    cross-engine edge be a standalone wait_ge.
    """
    nchunk = pairs // chunk_pairs
    u_per_chunk = chunk_pairs // 128
    fw = 16 * u_per_chunk  # psum/free width per chunk (512 for default)
    assert nchunk * fw <= 4096, "psum overflow"
    nc = bass.Bass()
    a_d = nc.declare_dram_parameter("a", [pairs, 128], BF16, isOutput=False)
    b_d = nc.declare_dram_parameter("b", [pairs, 128], BF16, isOutput=False)
    pw_d = nc.declare_dram_parameter("pw", [32, 128], BF16, isOutput=False)
    w_d = nc.declare_dram_parameter("w", [fw, 128], BF16, isOutput=False)
    out_d = nc.declare_dram_parameter(
        "out", [nchunk, 2 * chunk_pairs], I32, isOutput=True
    )

    from contextlib import ExitStack

    with ExitStack() as ctx:
        ent = ctx.enter_context
        a_sb = ent(nc.sbuf_tensor("a_sb", [128, pairs], BF16))
        b_sb = ent(nc.sbuf_tensor("b_sb", [128, pairs], BF16))
        pw_sb = ent(nc.sbuf_tensor("pw_sb", [128, 32], BF16))
        w_sb = ent(nc.sbuf_tensor("w_sb", [128, fw], BF16))
        bt = ent(nc.sbuf_tensor("bt", [128, fw], F32))
        g = ent(nc.sbuf_tensor("g", [128, fw], F32))
        m = ent(nc.sbuf_tensor("m", [128, fw], F32))
        bf = ent(nc.sbuf_tensor("bf", [128, fw // 8], F32))
        fence_sb = ent(nc.sbuf_tensor("fence_sb", [128, 2], BF16))
        bi = ent(nc.sbuf_tensor("bi", [128, nchunk * fw // 8], I32))
        ps = ent(nc.psum_tensor("ps", [128, nchunk * fw], F32))

        cs_sem = ent(nc.semaphore("cs_sem"))
        ch_sems = [ent(nc.semaphore(f"ch_sem{c}")) for c in range(nchunk)]
        mm_sem = ent(nc.semaphore("mm_sem"))
        dve_sem = ent(nc.semaphore("dve_sem"))
        out_sem = ent(nc.semaphore("out_sem"))

        with nc.Block() as block:

            @block.sync
            def _(sync):
                sync.dma_start(
                    out=pw_sb[:], in_=pw_d[:], transpose=True
                ).then_inc(cs_sem, 16)
                sync.dma_start(
                    out=w_sb[:], in_=w_d[:], transpose=True
                ).then_inc(cs_sem, 16)
                for c in range(nchunk):
                    sl = slice(c * chunk_pairs, (c + 1) * chunk_pairs)
                    sync.dma_start(
                        out=a_sb[:, sl], in_=a_d[sl, :], transpose=True
                    ).then_inc(ch_sems[c], 16)
                    sync.dma_start(
                        out=b_sb[:, sl], in_=b_d[sl, :], transpose=True
                    ).then_inc(ch_sems[c], 16)
                # outputs: only after ALL compute (also keeps the xbar
                # transpose mode quiesced before normal DMAs run)
                sync.wait_ge(dve_sem, nchunk)
                for c in range(nchunk):
                    sync.dma_start(
                        out=out_d[c].rearrange("(u p j) -> p u j", p=128, j=2),
                        in_=bi[:, c * fw // 8 : (c + 1) * fw // 8].rearrange(
                            "p (u j) -> p u j", j=2
                        ),
                    ).then_inc(out_sem, 16)
                sync.wait_ge(out_sem, 16 * nchunk)

            @block.tensor
            def _(tensor):
                tensor.wait_ge(cs_sem, 32)
                for c in range(nchunk):
                    tensor.wait_ge(ch_sems[c], 32)
                    for u in range(u_per_chunk):
                        col = c * chunk_pairs + u * 128
                        lhsA = a_sb[:, col : col + 128]
                        lhsB = b_sb[:, col : col + 128]
                        o = ps[:, c * fw + u * 16 : c * fw + (u + 1) * 16]
                        tensor.matmul(
                            o, lhsA, pw_sb[:, 0:16], start=True, stop=False
                        )
                        tensor.matmul(
                            o, lhsA, pw_sb[:, 16:32], start=False, stop=False
                        )
                        mm3 = tensor.matmul(
                            o, lhsB, pw_sb[:, 0:16], start=False, stop=True
                        )
                    mm3.then_inc(mm_sem, 1)

            @block.vector
            def _(vector):
                vector.wait_ge(cs_sem, 32)
                for c in range(nchunk):
                    vector.wait_ge(mm_sem, c + 1)
                    psl = ps[:, c * fw : (c + 1) * fw]
                    vector.tensor_single_scalar(
                        bt[:], psl, 0.0, mybir.AluOpType.is_gt
                    )
                    if gray:
                        vector.tensor_tensor(
                            g[:, 0 : fw - 1],
                            bt[:, 0 : fw - 1],
                            bt[:, 1:fw],
                            op=mybir.AluOpType.not_equal,
                        )
                        vector.tensor_copy(g[:, 7:fw:8], bt[:, 7:fw:8])
                        msrc = g
                    else:
                        msrc = bt
                    vector.tensor_mul(m[:], msrc[:], w_sb[:])
                    vector.tensor_reduce(
                        bf[:],
                        m[:].rearrange("p (k r) -> p k r", r=8),
                        axis=mybir.AxisListType.X,
                        op=mybir.AluOpType.add,
                    )
                    vector.tensor_copy(
                        bi[:, c * fw // 8 : (c + 1) * fw // 8], bf[:]
                    ).then_inc(dve_sem, 1)
    return nc


def _build(gray: bool, pairs: int = PAIRS, chunk_pairs: int = CHUNK_PAIRS):
    nchunk = pairs // chunk_pairs
    u_per_chunk = chunk_pairs // 128
    nc = bass.Bass()
    a_d = nc.declare_dram_parameter("a", [pairs, 128], BF16, isOutput=False)
    b_d = nc.declare_dram_parameter("b", [pairs, 128], BF16, isOutput=False)
    pw_d = nc.declare_dram_parameter("pw", [32, 128], BF16, isOutput=False)
    w_d = nc.declare_dram_parameter("w", [512, 128], BF16, isOutput=False)
    out_d = nc.declare_dram_parameter(
        "out", [nchunk, 2 * chunk_pairs], I32, isOutput=True
    )

    bis = []
    with tile.TileContext(nc) as tc:
        with (
            tc.tile_pool(name="const", bufs=1) as constp,
            tc.tile_pool(name="ab", bufs=8) as abp,
            tc.tile_pool(name="work", bufs=2) as workp,
            tc.tile_pool(name="outp", bufs=8) as outp,
            tc.tile_pool(name="psum", bufs=8, space=bass.MemorySpace.PSUM) as psump,
        ):
            pw_sb = constp.tile([128, 32], BF16)
            nc.sync.dma_start(out=pw_sb[:], in_=pw_d[:], transpose=True)
            w_sb = constp.tile([128, 512], BF16)
            nc.sync.dma_start(out=w_sb[:], in_=w_d[:], transpose=True)
            # Touch w_sb on DVE once so later tensor_tensor ops don't each
            # need a DMA-queue wait (the DVE TT ISA slot allows only one
            # sync wait; walrus rejects two).
            w_touch = constp.tile([128, 1], F32)  # dtype-cast copy is fine
            nc.vector.tensor_copy(w_touch[:], w_sb[:, 0:1])

            for c in range(nchunk):
                sl = slice(c * chunk_pairs, (c + 1) * chunk_pairs)
                a_sb = abp.tile([128, chunk_pairs], BF16, tag="a")
                nc.sync.dma_start(out=a_sb[:], in_=a_d[sl, :], transpose=True)
                b_sb = abp.tile([128, chunk_pairs], BF16, tag="b")
                nc.sync.dma_start(out=b_sb[:], in_=b_d[sl, :], transpose=True)

                fw = 16 * u_per_chunk
                ps = psump.tile([128, fw], F32)
                for u in range(u_per_chunk):
                    lhsA = a_sb[:, u * 128 : (u + 1) * 128]
                    lhsB = b_sb[:, u * 128 : (u + 1) * 128]
                    o = ps[:, u * 16 : (u + 1) * 16]
                    nc.tensor.matmul(o, lhsA, pw_sb[:, 0:16], start=True, stop=False)
                    nc.tensor.matmul(o, lhsA, pw_sb[:, 16:32], start=False, stop=False)
                    nc.tensor.matmul(o, lhsB, pw_sb[:, 0:16], start=False, stop=True)

                bt = workp.tile([128, fw], F32, tag="bt")
                nc.vector.tensor_single_scalar(
                    bt[:], ps[:], 0.0, mybir.AluOpType.is_gt
                )
                if gray:
                    g = workp.tile([128, fw], F32, tag="g")
                    nc.vector.tensor_tensor(
                        g[:, 0 : fw - 1],
                        bt[:, 0 : fw - 1],
                        bt[:, 1:fw],
                        op=mybir.AluOpType.not_equal,
                    )
                    nc.vector.tensor_copy(g[:, 7:fw:8], bt[:, 7:fw:8])
                    msrc = g
                else:
                    msrc = bt
                m = workp.tile([128, fw], F32, tag="m")
                nc.vector.tensor_mul(m[:], msrc[:], w_sb[:, 0:fw])
                bf = workp.tile([128, fw // 8], F32, tag="bf")
                nc.vector.tensor_reduce(
                    bf[:],
                    m[:].rearrange("p (k r) -> p k r", r=8),
                    axis=mybir.AxisListType.X,
                    op=mybir.AluOpType.add,
                )
                bi = outp.tile([128, fw // 8], I32, tag=f"bi{c}")
                nc.vector.tensor_copy(bi[:], bf[:])
                bis.append(bi)
    # Past the TileContext exit: Tile has drained every engine and run an
    # all-engine barrier, so raw output DMAs here need no sync waits at
    # all (the DMA ISA slot only has one wait; inside the context the
    # xbar-transpose serialization would force 2+). Fence completion with
    # one semaphore.
    post = []
    with nc.semaphore("out_sem") as out_sem:
        for c, bi in enumerate(bis):
            h = nc.sync.dma_start(
                out=out_d[c].rearrange("(u p j) -> p u j", p=128, j=2),
                in_=bi[:].rearrange("p (u j) -> p u j", j=2),
            )
            h.then_inc(out_sem, 16)
            post.append(h)
        nc.sync.wait_ge(out_sem, 16 * len(bis))

    # Tile lowered only the instructions it traced; resolve the symbolic
    # tile APs on the raw post-context DMAs the same way tile.py does.
    def _concrete(arg):
        t = arg.bass_ap.tensor
        if hasattr(t, "concrete_tensor"):
            arg.bass_ap.tensor = t.concrete_tensor()
        return arg.bass_ap

    eng = nc.sync
    for h in post:
        inst = h.ins
        inst.ins, inst.outs = eng.lower_symbolic_args(
            inst.ins, inst.outs, _concrete, inst.debug
        )
    return nc


def _prep(mat, proj_dir, perm, enc_vec, gray):
    bf16 = ml_dtypes.bfloat16
    flat = np.ascontiguousarray(mat.reshape(B * H, N, D), dtype=np.float32)
    a_full = flat.astype(bf16)
    b_full = (flat - a_full.astype(np.float32)).astype(bf16)

    p = np.asarray(proj_dir, dtype=np.float32).reshape(D, NPROJ)
    pa = p.astype(bf16)
    pb = (p - pa.astype(np.float32)).astype(bf16)
    pw = np.zeros((128, 32), dtype=bf16)
    pw[0:64, 0:8] = pa
    pw[64:128, 8:16] = pa
    pw[0:64, 16:24] = pb
    pw[64:128, 24:32] = pb
    pw = np.ascontiguousarray(pw.T)  # shipped transposed; X-bar restores

    enc = np.asarray(enc_vec, dtype=np.float32).reshape(NPROJ)
    w = np.tile(enc, 64).reshape(1, 512).repeat(128, axis=0)
    if gray:
        # weights for the XOR-of-adjacent-bits formulation: 2^r for r<7,
        # 128 on the patched r=7 slot -- identical to enc for enc=2^r.
        w = np.tile(
            np.array([1, 2, 4, 8, 16, 32, 64, 128], dtype=np.float32), 64
        ).reshape(1, 512).repeat(128, axis=0)
    w = np.ascontiguousarray(w.T, dtype=np.float32).astype(bf16)

    bh_per_core = B * H // N_CORES
    in_maps = []
    for i in range(N_CORES):
        sh = a_full[i * bh_per_core : (i + 1) * bh_per_core]
        shb = b_full[i * bh_per_core : (i + 1) * bh_per_core]
        in_maps.append(
            {
                "a": np.ascontiguousarray(sh).reshape(PAIRS, 128),
                "b": np.ascontiguousarray(shb).reshape(PAIRS, 128),
                "pw": pw,
                "w": w,
            }
        )
    return in_maps



def _build_v4(gray: bool, pairs: int = PAIRS, chunk_pairs: int = CHUNK_PAIRS):
    """a-only variant of the xbar kernel: halves input DMA (the packet-rate
    bottleneck). Emits per-row min|score| so the host exactly recomputes
    rows inside the bf16 rounding envelope."""
    nchunk = pairs // chunk_pairs
    u_per_chunk = chunk_pairs // 128
    fw = 16 * u_per_chunk
    assert nchunk * fw <= 4096, "psum overflow"
    nc = bass.Bass()
    a_d = nc.declare_dram_parameter("a", [pairs, 128], BF16, isOutput=False)
    pw_d = nc.declare_dram_parameter("pw", [32, 128], BF16, isOutput=False)
    w_d = nc.declare_dram_parameter("w", [fw, 128], BF16, isOutput=False)
    out_d = nc.declare_dram_parameter(
        "out", [nchunk, 2 * chunk_pairs], I32, isOutput=True
    )

    from contextlib import ExitStack

    with ExitStack() as ctx:
        ent = ctx.enter_context
        a_sb = ent(nc.sbuf_tensor("a_sb", [128, pairs], BF16))
        pw_sb = ent(nc.sbuf_tensor("pw_sb", [128, 32], BF16))
        w_sb = ent(nc.sbuf_tensor("w_sb", [128, fw], BF16))
        bt = ent(nc.sbuf_tensor("bt", [128, fw], F32))
        g = ent(nc.sbuf_tensor("g", [128, fw], F32))
        m = ent(nc.sbuf_tensor("m", [128, fw], F32))
        bf = ent(nc.sbuf_tensor("bf", [128, fw // 8], F32))
        flg = ent(nc.sbuf_tensor("flg", [128, fw // 8], F32))
        bi = ent(nc.sbuf_tensor("bi", [128, nchunk * fw // 8], I32))
        ma_sb = ent(nc.sbuf_tensor("ma_sb", [128, nchunk * fw // 8], F32))
        ps = ent(nc.psum_tensor("ps", [128, nchunk * fw], F32))

        cs_sem = ent(nc.semaphore("cs_sem"))
        ch_sems = [ent(nc.semaphore(f"ch_sem{c}")) for c in range(nchunk)]
        mm_sem = ent(nc.semaphore("mm_sem"))
        dve_sem = ent(nc.semaphore("dve_sem"))
        out_sem = ent(nc.semaphore("out_sem"))

        with nc.Block() as block:

            @block.sync
            def _(sync):
                sync.dma_start(
                    out=pw_sb[:], in_=pw_d[:], transpose=True
                ).then_inc(cs_sem, 16)
                sync.dma_start(
                    out=w_sb[:], in_=w_d[:], transpose=True
                ).then_inc(cs_sem, 16)
                for c in range(nchunk):
                    sl = slice(c * chunk_pairs, (c + 1) * chunk_pairs)
                    sync.dma_start(
                        out=a_sb[:, sl], in_=a_d[sl, :], transpose=True
                    ).then_inc(ch_sems[c], 16)
                sync.wait_ge(dve_sem, nchunk)
                for c in range(nchunk):
                    csl = slice(c * fw // 8, (c + 1) * fw // 8)
                    sync.dma_start(
                        out=out_d[c].rearrange("(u p j) -> p u j", p=128, j=2),
                        in_=bi[:, csl].rearrange("p (u j) -> p u j", j=2),
                    ).then_inc(out_sem, 16)
                sync.wait_ge(out_sem, 16 * nchunk)

            @block.tensor
            def _(tensor):
                tensor.wait_ge(cs_sem, 32)
                for c in range(nchunk):
                    tensor.wait_ge(ch_sems[c], 16)
                    for u in range(u_per_chunk):
                        col = c * chunk_pairs + u * 128
                        lhsA = a_sb[:, col : col + 128]
                        o = ps[:, c * fw + u * 16 : c * fw + (u + 1) * 16]
                        tensor.matmul(
                            o, lhsA, pw_sb[:, 0:16], start=True, stop=False
                        )
                        mm = tensor.matmul(
                            o, lhsA, pw_sb[:, 16:32], start=False, stop=True
                        )
                    mm.then_inc(mm_sem, 1)

            @block.vector
            def _(vector):
                vector.wait_ge(cs_sem, 32)
                for c in range(nchunk):
                    vector.wait_ge(mm_sem, c + 1)
                    psl = ps[:, c * fw : (c + 1) * fw]
                    csl = slice(c * fw // 8, (c + 1) * fw // 8)
                    vector.tensor_single_scalar(
                        bt[:], psl, 0.0, mybir.AluOpType.is_gt
                    )
                    if gray:
                        vector.tensor_tensor(
                            g[:, 0 : fw - 1],
                            bt[:, 0 : fw - 1],
                            bt[:, 1:fw],
                            op=mybir.AluOpType.not_equal,
                        )
                        vector.tensor_copy(g[:, 7:fw:8], bt[:, 7:fw:8])
                        msrc = g
                    else:
                        msrc = bt
                    vector.tensor_mul(m[:], msrc[:], w_sb[:])
                    vector.tensor_reduce(
                        bf[:],
                        m[:].rearrange("p (k r) -> p k r", r=8),
                        axis=mybir.AxisListType.X,
                        op=mybir.AluOpType.add,
                    )
                    vector.tensor_reduce(
                        ma_sb[:, csl],
                        psl.rearrange("p (k r) -> p k r", r=8),
                        axis=mybir.AxisListType.X,
                        op=mybir.AluOpType.min,
                        apply_absolute_value=True,
                    )
                    # fuse the "needs host recompute" flag into bit 8 of the
                    # bucket word: out = bucket + 256*(min|score| < TAU)
                    vector.tensor_scalar(
                        flg[:], ma_sb[:, csl], TAU, 256.0,
                        mybir.AluOpType.is_lt, mybir.AluOpType.mult,
                    )
                    vector.tensor_add(bf[:], bf[:], flg[:])
                    vector.tensor_copy(bi[:, csl], bf[:]).then_inc(dve_sem, 1)
    return nc


def _prep_v4(mat, proj_dir, enc_vec, gray):
    bf16 = ml_dtypes.bfloat16
    flat = np.ascontiguousarray(mat.reshape(B * H, N, D), dtype=np.float32)
    a_full = flat.astype(bf16)

    p = np.asarray(proj_dir, dtype=np.float32).reshape(D, NPROJ)
    pa = p.astype(bf16)
    pb = (p - pa.astype(np.float32)).astype(bf16)
    pw = np.zeros((128, 32), dtype=bf16)
    pw[0:64, 0:8] = pa
    pw[64:128, 8:16] = pa
    pw[0:64, 16:24] = pb
    pw[64:128, 24:32] = pb
    pw = np.ascontiguousarray(pw.T)

    wrow = (
        np.array([1, 2, 4, 8, 16, 32, 64, 128], dtype=np.float32)
        if gray
        else np.asarray(enc_vec, dtype=np.float32).reshape(NPROJ)
    )
    w = np.tile(wrow, 64).reshape(1, 512).repeat(128, axis=0)
    w = np.ascontiguousarray(w.T).astype(bf16)

    bh_per_core = B * H // N_CORES
    in_maps = []
    for i in range(N_CORES):
        sh = a_full[i * bh_per_core : (i + 1) * bh_per_core]
        in_maps.append(
            {
                "a": np.ascontiguousarray(sh).reshape(PAIRS, 128),
                "pw": pw,
                "w": w,
            }
        )
    return in_maps


def _build_v5(gray: bool, pairs: int = PAIRS, chunk_pairs: int = CHUNK_PAIRS):
    """Natural full-bandwidth loads of a HOST-pre-transposed bf16 image.

    The host ships a^T as [128, pairs] (packed-d on partitions), so every
    input DMA is a plain contiguous descriptor (8 KiB/partition) that runs
    at the full ~360 GB/s fabric rate — no X-bar transpose (132 B packets,
    measured 0.26% MBU) and no PE transpose pass.  Pipeline per chunk:
    PE matmul (data stationary, pw streamed, hi+lo proj accumulated in
    PSUM) -> ACT evacuates PSUM to bf16 SBUF -> DVE computes Gray bucket
    bits + the min|score| "host must recompute" flag in bf16 (2x rate)
    -> SP DMAs the fused bucket|flag word out.
    """
    nchunk = pairs // chunk_pairs
    u_per_chunk = chunk_pairs // 128
    fw = 16 * u_per_chunk  # psum/free width per chunk (512 for default)
    assert nchunk * fw <= 4096, "psum overflow"
    nc = bass.Bass()
    a_d = nc.declare_dram_parameter("a", [128, pairs], BF16, isOutput=False)
    # constants fused into one DMA: cols 0:32 = pw, 32:32+fw = w
    cst_d = nc.declare_dram_parameter("cst", [128, 32 + fw], BF16, isOutput=False)
    out_d = nc.declare_dram_parameter(
        "out", [128, nchunk * fw // 8], I32, isOutput=True
    )

    from contextlib import ExitStack

    with ExitStack() as ctx:
        ent = ctx.enter_context
        a_sb = ent(nc.sbuf_tensor("a_sb", [128, pairs], BF16))
        cst_sb = ent(nc.sbuf_tensor("cst_sb", [128, 32 + fw], BF16))
        pw_sb = cst_sb[:, 0:32]
        w_sb = cst_sb[:, 32 : 32 + fw]
        s16 = ent(nc.sbuf_tensor("s16", [128, nchunk * fw], BF16))
        bt = ent(nc.sbuf_tensor("bt", [128, fw], BF16))
        g = ent(nc.sbuf_tensor("g", [128, fw], BF16))
        m = ent(nc.sbuf_tensor("m", [128, fw], BF16))
        bf = ent(nc.sbuf_tensor("bf", [128, fw // 8], F32))
        ma_sb = ent(nc.sbuf_tensor("ma_sb", [128, fw // 8], BF16))
        flg = ent(nc.sbuf_tensor("flg", [128, fw // 8], F32))
        bi = ent(nc.sbuf_tensor("bi", [128, nchunk * fw // 8], I32))
        ps = ent(nc.psum_tensor("ps", [128, nchunk * fw], F32))

        cs_sem = ent(nc.semaphore("cs_sem"))
        ch_sems = [ent(nc.semaphore(f"ch_sem{c}")) for c in range(nchunk)]
        mm_sem = ent(nc.semaphore("mm_sem"))
        act_sem = ent(nc.semaphore("act_sem"))
        dve_sem = ent(nc.semaphore("dve_sem"))
        out_sem = ent(nc.semaphore("out_sem"))

        with nc.Block() as block:

            @block.sync
            def _(sync):
                for c in range(nchunk):
                    sl = slice(c * chunk_pairs, (c + 1) * chunk_pairs)
                    sync.dma_start(out=a_sb[:, sl], in_=a_d[:, sl]).then_inc(
                        ch_sems[c], 16
                    )
                sync.wait_ge(out_sem, 32)

            @block.tensor
            def _(tensor):
                tensor.wait_ge(cs_sem, 16)
                for c in range(nchunk):
                    tensor.wait_ge(ch_sems[c], 16)
                    for i in range(u_per_chunk):
                        u = c * u_per_chunk + i
                        col = u * 128
                        lhsA = a_sb[:, col : col + 128]
                        o = ps[:, u * 16 : (u + 1) * 16]
                        tensor.matmul(
                            o, lhsA, pw_sb[:, 0:16], start=True, stop=False
                        )
                        mm = tensor.matmul(
                            o, lhsA, pw_sb[:, 16:32], start=False, stop=True
                        )
                    mm.then_inc(mm_sem, 1)

            @block.scalar
            def _(scalar):
                scalar.dma_start(out=cst_sb[:], in_=cst_d[:]).then_inc(
                    cs_sem, 16
                )
                for c in range(nchunk):
                    scalar.wait_ge(mm_sem, c + 1)
                    psl = slice(c * fw, (c + 1) * fw)
                    scalar.copy(s16[:, psl], ps[:, psl]).then_inc(act_sem, 1)
                half = nchunk * fw // 16
                scalar.wait_ge(dve_sem, nchunk // 2)
                scalar.dma_start(
                    out=out_d[:, 0:half], in_=bi[:, 0:half]
                ).then_inc(out_sem, 16)
                scalar.wait_ge(dve_sem, nchunk)
                scalar.dma_start(
                    out=out_d[:, half : 2 * half], in_=bi[:, half : 2 * half]
                ).then_inc(out_sem, 16)

            @block.vector
            def _(vector):
                vector.wait_ge(cs_sem, 16)
                for c in range(nchunk):
                    vector.wait_ge(act_sem, c + 1)
                    ssl = s16[:, c * fw : (c + 1) * fw]
                    csl = slice(c * fw // 8, (c + 1) * fw // 8)
                    vector.tensor_single_scalar(
                        bt[:], ssl, 0.0, mybir.AluOpType.is_gt
                    )
                    if gray:
                        vector.tensor_tensor(
                            g[:, 0 : fw - 1],
                            bt[:, 0 : fw - 1],
                            bt[:, 1:fw],
                            op=mybir.AluOpType.not_equal,
                        )
                        vector.tensor_copy(g[:, 7:fw:8], bt[:, 7:fw:8])
                        msrc = g
                    else:
                        msrc = bt
                    vector.tensor_mul(m[:], msrc[:], w_sb[:])
                    vector.tensor_reduce(
                        bf[:],
                        m[:].rearrange("p (k r) -> p k r", r=8),
                        axis=mybir.AxisListType.X,
                        op=mybir.AluOpType.add,
                    )
                    vector.tensor_reduce(
                        ma_sb[:],
                        ssl.rearrange("p (k r) -> p k r", r=8),
                        axis=mybir.AxisListType.X,
                        op=mybir.AluOpType.min,
                        apply_absolute_value=True,
                    )
                    # fuse the "needs host recompute" flag into bit 8 of the
                    # bucket word: out = bucket + 256*(min|score| < TAU)
                    vector.tensor_scalar(
                        flg[:], ma_sb[:], TAU, 256.0,
                        mybir.AluOpType.is_lt, mybir.AluOpType.mult,
                    )
                    vector.tensor_add(bf[:], bf[:], flg[:])
                    vector.tensor_copy(bi[:, csl], bf[:]).then_inc(dve_sem, 1)
    return nc


def _prep_v5(mat, proj_dir, enc_vec, gray):
    bf16 = ml_dtypes.bfloat16
    flat = np.ascontiguousarray(mat.reshape(B * H, N, D), dtype=np.float32)
    a_full = flat.astype(bf16)

    p = np.asarray(proj_dir, dtype=np.float32).reshape(D, NPROJ)
    pa = p.astype(bf16)
    pb = (p - pa.astype(np.float32)).astype(bf16)
    pw = np.zeros((128, 32), dtype=bf16)
    pw[0:64, 0:8] = pa
    pw[64:128, 8:16] = pa
    pw[0:64, 16:24] = pb
    pw[64:128, 24:32] = pb

    wrow = (
        np.array([1, 2, 4, 8, 16, 32, 64, 128], dtype=np.float32)
        if gray
        else np.asarray(enc_vec, dtype=np.float32).reshape(NPROJ)
    )
    w = np.tile(wrow, 64).reshape(1, 512).repeat(128, axis=0).astype(bf16)
    cst = np.ascontiguousarray(np.concatenate([pw, w], axis=1))  # [128, 544]

    bh_per_core = B * H // N_CORES
    in_maps = []
    for i in range(N_CORES):
        sh = a_full[i * bh_per_core : (i + 1) * bh_per_core]
        at = np.ascontiguousarray(sh.reshape(PAIRS, 128).T)  # [128, PAIRS]
        in_maps.append({"a": at, "cst": cst})
    return in_maps


def _build_v7(
    gray: bool,
    pairs: int = PAIRS,
    chunk_pairs: int = CHUNK_PAIRS,
    fence: bool = False,
    act_sign: bool = True,
    split_in: bool = True,
    coarse: bool = False,
    no_ma: bool = False,
    mm_inc_fine: bool = True,
    _no_races: bool = True,
):
    """v6 + hazard-free epilogue and leaner engine split.

    Correctness: v6's only failure was the strided sub-word write
    (g[7::8] bf16, 2B stores every 16B) whose RMW drain the immediately
    following reader could overtake -> nondeterministic +-128 bucket
    errors. v7 keeps every DVE WRITE contiguous (strided 3D reads are
    fine) and adds v3's ACT fence (re-read the tail on the same engine
    before the semaphore inc).

    Split: ACT extracts sign(score) from PSUM directly (Sign activation,
    bf16 out); DVE per halfchunk does just 6 contiguous-write ops:
    ne(sign_r, sign_{r+1}) -> 7-wide weighted reduce -> +128*b7 term,
    plus min|score| straight from PSUM. Bucket word ships as f32 and
    min|score| as raw bf16 (host thresholds + casts). Input DMAs split
    across both HWDGE queues (sync + scalar).
    """
    nchunk = pairs // chunk_pairs
    u_per_chunk = chunk_pairs // 128
    fw = 16 * u_per_chunk  # psum cols per chunk (512)
    hdiv = 1 if coarse else 2
    nh = hdiv * nchunk     # ACT/DVE halfchunks
    fh = fw // hdiv        # psum cols per halfchunk
    kh = fh // 8           # rows (bucket words) per partition per halfchunk
    assert nchunk * fw <= 4096, "psum overflow"
    nc = bass.Bass(detect_race_conditions=_no_races)
    a_d = nc.declare_dram_parameter("a", [128, pairs], BF16, isOutput=False)
    # constants: cols 0:32 = pw, 32:32+7*kh = w7 (tiled [1,2,...,64])
    cw = 32 + 7 * kh
    cst_d = nc.declare_dram_parameter("cst", [128, cw], BF16, isOutput=False)
    wd_d = nc.declare_dram_parameter("wd", [128, nh * kh], F32, isOutput=True)
    ma_d = nc.declare_dram_parameter("ma", [128, nh * kh], BF16, isOutput=True)

    from contextlib import ExitStack

    with ExitStack() as ctx:
        ent = ctx.enter_context
        a_sb = ent(nc.sbuf_tensor("a_sb", [128, pairs], BF16))
        cst_sb = ent(nc.sbuf_tensor("cst_sb", [128, cw], BF16))
        pw_sb = cst_sb[:, 0:32]
        w7_sb = cst_sb[:, 32 : 32 + 7 * kh]
        sg = ent(nc.sbuf_tensor("sg", [128, nh * fh], BF16))
        bt = ent(nc.sbuf_tensor("bt", [128, fh], BF16))
        g = ent(nc.sbuf_tensor("g", [128, 7 * kh], BF16))
        m = ent(nc.sbuf_tensor("m", [128, 7 * kh], BF16))
        bf = ent(nc.sbuf_tensor("bf", [128, kh], F32))
        b7c = ent(nc.sbuf_tensor("b7c", [128, kh], F32))
        wd = ent(nc.sbuf_tensor("wd_sb", [128, nh * kh], F32))
        ma_sb = ent(nc.sbuf_tensor("ma_sb", [128, nh * kh], BF16))
        fence_sb = ent(nc.sbuf_tensor("fence_sb", [128, 2], BF16))
        ps = ent(nc.psum_tensor("ps", [128, nchunk * fw], F32))

        cs_sem = ent(nc.semaphore("cs_sem"))
        ch_sems = [ent(nc.semaphore(f"ch_sem{c}")) for c in range(nchunk)]
        mm_sem = ent(nc.semaphore("mm_sem"))
        act_sem = ent(nc.semaphore("act_sem"))
        dve_sem = ent(nc.semaphore("dve_sem"))
        out_sem = ent(nc.semaphore("out_sem"))

        with nc.Block() as block:

            @block.sync
            def _(sync):
                step = 2 if split_in else 1
                for c in range(0, nchunk, step):
                    sl = slice(c * chunk_pairs, (c + 1) * chunk_pairs)
                    sync.dma_start(out=a_sb[:, sl], in_=a_d[:, sl]).then_inc(
                        ch_sems[c], 16
                    )
                sync.wait_ge(out_sem, 32 if no_ma else 64)

            @block.tensor
            def _(tensor):
                tensor.wait_ge(cs_sem, 16)
                for c in range(nchunk):
                    tensor.wait_ge(ch_sems[c], 16)
                    for i in range(u_per_chunk):
                        u = c * u_per_chunk + i
                        col = u * 128
                        lhsA = a_sb[:, col : col + 128]
                        o = ps[:, u * 16 : (u + 1) * 16]
                        tensor.matmul(
                            o, lhsA, pw_sb[:, 0:16], start=True, stop=False
                        )
                        mm = tensor.matmul(
                            o, lhsA, pw_sb[:, 16:32], start=False, stop=True
                        )
                        step = u_per_chunk // (hdiv if mm_inc_fine else 1)
                        if i % step == step - 1:
                            mm.then_inc(mm_sem, 1)

            @block.scalar
            def _(scalar):
                scalar.dma_start(out=cst_sb[:], in_=cst_d[:]).then_inc(
                    cs_sem, 16
                )
                if split_in:
                    for c in range(1, nchunk, 2):
                        sl = slice(c * chunk_pairs, (c + 1) * chunk_pairs)
                        scalar.dma_start(
                            out=a_sb[:, sl], in_=a_d[:, sl]
                        ).then_inc(ch_sems[c], 16)
                for h in range(nh):
                    scalar.wait_ge(
                        mm_sem, h + 1 if mm_inc_fine else h // hdiv + 1
                    )
                    hsl = slice(h * fh, (h + 1) * fh)
                    if act_sign:
                        op = scalar.sign(sg[:, hsl], ps[:, hsl])
                    else:
                        op = scalar.copy(sg[:, hsl], ps[:, hsl])
                    if fence:
                        # fence: re-read the copy tail on the same engine so
                        # the inc cannot outrun the final SBUF writes
                        op = scalar.copy(
                            fence_sb[:], sg[:, (h + 1) * fh - 2 : (h + 1) * fh]
                        )
                    op.then_inc(act_sem, 1)
                half = nh * kh // 2
                scalar.wait_ge(dve_sem, nh // 2)
                scalar.dma_start(
                    out=wd_d[:, 0:half], in_=wd[:, 0:half]
                ).then_inc(out_sem, 16)
                if not no_ma:
                    scalar.dma_start(
                        out=ma_d[:, 0:half], in_=ma_sb[:, 0:half]
                    ).then_inc(out_sem, 16)
                scalar.wait_ge(dve_sem, nh)
                scalar.dma_start(
                    out=wd_d[:, half : 2 * half], in_=wd[:, half : 2 * half]
                ).then_inc(out_sem, 16)
                if not no_ma:
                    scalar.dma_start(
                        out=ma_d[:, half : 2 * half],
                        in_=ma_sb[:, half : 2 * half],
                    ).then_inc(out_sem, 16)

            @block.vector
            def _(vector):
                vector.wait_ge(cs_sem, 16)
                for h in range(nh):
                    vector.wait_ge(act_sem, h + 1)
                    hs = slice(h * kh, (h + 1) * kh)
                    sgh = sg[:, h * fh : (h + 1) * fh].rearrange(
                        "p (k r) -> p k r", r=8
                    )
                    psh = ps[:, h * fh : (h + 1) * fh].rearrange(
                        "p (k r) -> p k r", r=8
                    )
                    g3 = g[:].rearrange("p (k r) -> p k r", r=7)
                    m3 = m[:].rearrange("p (k r) -> p k r", r=7)
                    if act_sign:
                        bth = sgh  # ACT shipped sign(score) in {-1,0,1}
                        b7_scale, b7_bias = 64.0, 64.0
                    else:
                        bt3 = bt[:].rearrange("p (k r) -> p k r", r=8)
                        vector.tensor_single_scalar(
                            bt[:],
                            sg[:, h * fh : (h + 1) * fh],
                            0.0,
                            mybir.AluOpType.is_gt,
                        )
                        bth = bt3
                        b7_scale, b7_bias = 128.0, 0.0
                    if gray:
                        vector.tensor_tensor(
                            g3,
                            bth[:, :, 0:7],
                            bth[:, :, 1:8],
                            op=mybir.AluOpType.not_equal,
                        )
                    else:
                        # raw bin ids: bit r is just (sign_r > 0)
                        vector.tensor_scalar(
                            g3, bth[:, :, 0:7], 0.0, 1.0,
                            mybir.AluOpType.is_gt, mybir.AluOpType.mult,
                        )
                    vector.tensor_mul(m[:], g[:], w7_sb[:])
                    vector.tensor_reduce(
                        bf[:],
                        m3,
                        axis=mybir.AxisListType.X,
                        op=mybir.AluOpType.add,
                    )
                    # 128*b7 (sign==0 rows are flagged anyway)
                    vector.tensor_scalar(
                        b7c[:], bth[:, :, 7], b7_scale, b7_bias,
                        mybir.AluOpType.mult, mybir.AluOpType.add,
                    )
                    if not no_ma:
                        vector.tensor_reduce(
                            ma_sb[:, hs],
                            psh,
                            axis=mybir.AxisListType.X,
                            op=mybir.AluOpType.min,
                            apply_absolute_value=True,
                        )
                    vector.tensor_add(wd[:, hs], bf[:], b7c[:]).then_inc(
                        dve_sem, 1
                    )
    return nc


def _prep_v7(mat, proj_dir, enc_vec, gray):
    bf16 = ml_dtypes.bfloat16
    flat = np.ascontiguousarray(mat.reshape(B * H, N, D), dtype=np.float32)
    a_full = flat.astype(bf16)

    p = np.asarray(proj_dir, dtype=np.float32).reshape(D, NPROJ)
    pa = p.astype(bf16)
    pb = (p - pa.astype(np.float32)).astype(bf16)
    pw = np.zeros((128, 32), dtype=np.float32)
    pw[0:64, 0:8] = pa
    pw[64:128, 8:16] = pa
    pw[0:64, 16:24] = pb
    pw[64:128, 24:32] = pb

    kh = 32
    wrow = (
        np.array([1, 2, 4, 8, 16, 32, 64], dtype=np.float32)
        if gray
        else np.asarray(enc_vec, dtype=np.float32).reshape(NPROJ)[0:7]
    )
    w7 = np.tile(wrow, kh).reshape(1, 7 * kh).repeat(128, axis=0)
    cst = np.ascontiguousarray(
        np.concatenate([pw, w7], axis=1).astype(bf16)
    )  # [128, 32 + 7*kh]

    bh_per_core = B * H // N_CORES
    in_maps = []
    for i in range(N_CORES):
        sh = a_full[i * bh_per_core : (i + 1) * bh_per_core]
        at = np.ascontiguousarray(sh.reshape(PAIRS, 128).T)  # [128, PAIRS]
        in_maps.append({"a": at, "cst": cst})
    return in_maps


def _is_gray_setup(perm, enc_vec):
    perm = np.asarray(perm).reshape(-1)
    enc = np.asarray(enc_vec).reshape(-1)
    if perm.shape[0] != 256 or enc.shape[0] != NPROJ:
        return False
    idx = np.arange(256, dtype=np.int64)
    return bool(
        np.array_equal(perm, idx ^ (idx >> 1)) and np.array_equal(enc, 2 ** np.arange(8))
    )


def kernel(mat, proj_dir, perm, enc_vec, _trace=False, _tmpdir=None):
    gray = _is_gray_setup(perm, enc_vec)
    key = ("v7", gray)
    if key not in _cache:
        _cache[key] = _build_v7(gray, mm_inc_fine=False)
    nc = _cache[key]

    in_maps = _prep_v7(mat, proj_dir, enc_vec, gray)
    res = run_bass_kernel_spmd(
        nc, in_maps, core_ids=list(range(N_CORES)), trace=_trace, tmpdir=_tmpdir
    )

    # device outs are [128, 16*32]: col h*32 + u*2 + j of partition p is row
    # ((h*16 + u)*128 + p)*2 + j of the core's shard.
    def _decode(arr):
        return np.transpose(
            np.asarray(arr).reshape(128, 16, 16, 2), (1, 2, 0, 3)
        ).reshape(ROWS_PER_CORE)

    buckets = np.concatenate(
        [_decode(r["wd"]) for r in res.results]
    ).astype(np.int64)
    flagged = (
        np.concatenate(
            [_decode(r["ma"]).astype(np.float32) for r in res.results]
        )
        < TAU
    )

    # Host fix-up: rows whose smallest |bf16 score| is inside the rounding
    # envelope get recomputed exactly.
    flat = np.ascontiguousarray(mat.reshape(B * H * N, D), dtype=np.float32)
    p = np.asarray(proj_dir, dtype=np.float32).reshape(D, NPROJ)
    enc = np.asarray(enc_vec).reshape(NPROJ).astype(np.int64)
    perm_arr = np.asarray(perm).reshape(-1).astype(np.int64)
    if not gray:
        buckets = perm_arr[buckets]  # device emitted raw bin ids
    idx = np.nonzero(flagged)[0]
    if idx.size:
        sc = flat[idx] @ p
        bits = (sc > 0).astype(np.int64)
        bins = (bits * enc).sum(-1)
        buckets[idx] = (bins ^ (bins >> 1)) if gray else perm_arr[bins]
    out = buckets.reshape(B, H, N).astype(np.int32)
    if _trace:
        return out, res
    return out

